# revision 18
# baseline (speedup 1.0000x reference)
"""Trainium2 Bass kernel for nn_PolyAdapter (base Linear + LoRA + IA3 + dense top-1 MoE).

Math (per token row x of length 512):
  out = (x @ Wb.T + bb + 4*(x@A)@B) * ia3 + w_sel * (x @ We[sel].T + be[sel])
where sel = argmax softmax(x @ Wr.T + br), w_sel its softmax weight.

Host folds LoRA + IA3 into one base matrix:  V = (Wb.T + 4*A@B) * ia3[None,:],
bbp = bb*ia3.  Device computes, per 128-token tile:
  - PE transpose of x tile (fp32, exact) to get features-on-partitions
  - router logits in fp32 (exact argmax vs reference), softmax top-1 mask on DVE
  - base psum = x@V (+ bbp via ones-row matmul, + sum_e w_e*be[e] via wT matmul)
  - 4 expert matmuls (fp32r), combined with per-token weights via fused
    scalar_tensor_tensor ops on DVE.

Sharding: data-parallel, batch row b -> core b (8 cores, 8192 tokens each).
"""

import numpy as np

IN_F = 512
OUT_F = 512
RANK = 8
ALPHA = 32.0
N_EXP = 4
B = 8
S = 8192
P = 128
N_CORES = 8
NTILES = S // P  # 64 tiles of 128 tokens per core
KC = IN_F // P  # 4 contraction chunks

_compiled_nc = None


def _build(repeat=1, ablate=()):
    import concourse.bass as bass
    import concourse.tile as tile
    from concourse import bacc, mybir
    from concourse.masks import make_identity

    fp32 = mybir.dt.float32
    f32r = mybir.dt.float32r
    Alu = mybir.AluOpType
    Act = mybir.ActivationFunctionType
    Ax = mybir.AxisListType

    nc = bacc.Bacc("TRN2", target_bir_lowering=False, debug=False,
                   enable_asserts=False)

    # weights arrive pre-arranged in the exact SBUF layouts (see _prep_weights);
    # fp32r (tf32) operands are pre-rounded on the host
    x = nc.dram_tensor("x", [S, IN_F], fp32, kind="ExternalInput").ap()
    vt = nc.dram_tensor("vt", [P, KC, OUT_F], f32r, kind="ExternalInput").ap()
    bbp = nc.dram_tensor("bbp", [1, OUT_F], f32r, kind="ExternalInput").ap()
    wet = nc.dram_tensor("wet", [P, KC, N_EXP, OUT_F], f32r, kind="ExternalInput").ap()
    bep = nc.dram_tensor("bep", [N_EXP, OUT_F], f32r, kind="ExternalInput").ap()
    wrt = nc.dram_tensor("wrt", [P, KC, N_EXP], fp32, kind="ExternalInput").ap()
    brp = nc.dram_tensor("brp", [1, N_EXP], fp32, kind="ExternalInput").ap()
    ones = nc.dram_tensor("ones", [1, P], f32r, kind="ExternalInput").ap()
    out = nc.dram_tensor("out", [S, OUT_F], fp32, kind="ExternalOutput").ap()

    x_t = x.rearrange("(n p) i -> n p i", p=P)
    out_t = out.rearrange("(n p) o -> n p o", p=P)

    with tile.TileContext(nc) as tc:
        with (
            tc.tile_pool(name="singles", bufs=1) as singles,
            tc.tile_pool(name="xin", bufs=3) as xin,
            tc.tile_pool(name="xtp", bufs=2) as xtp,
            tc.tile_pool(name="smalls", bufs=2) as smalls,
            tc.tile_pool(name="outp", bufs=3) as outp,
            tc.tile_pool(name="ps_xt", bufs=1, space="PSUM") as ps_xt,
            tc.tile_pool(name="ps_sm", bufs=1, space="PSUM") as ps_sm,
            tc.tile_pool(name="ps_base", bufs=2, space="PSUM") as ps_base,
            tc.tile_pool(name="ps_exp", bufs=1, space="PSUM") as ps_exp,
        ):
            ident = singles.tile([P, P], fp32)
            make_identity(nc, ident)
            ones_r = singles.tile([1, P], f32r)
            nc.sync.dma_start(ones_r, ones)

            vt_sb = singles.tile([P, KC, OUT_F], f32r)
            nc.sync.dma_start(vt_sb, vt)
            wet_sb = singles.tile([P, KC, N_EXP, OUT_F], f32r)
            nc.sync.dma_start(wet_sb, wet)
            wrt_sb = singles.tile([P, KC, N_EXP], fp32)
            nc.sync.dma_start(wrt_sb, wrt)
            bbp_sb = singles.tile([1, OUT_F], f32r)
            nc.sync.dma_start(bbp_sb, bbp)
            bep_sb = singles.tile([N_EXP, OUT_F], f32r)
            nc.sync.dma_start(bep_sb, bep)
            brp_sb = singles.tile([P, N_EXP], fp32)
            nc.gpsimd.dma_start(
                brp_sb,
                bass.AP(tensor=brp.tensor, offset=brp.offset,
                        ap=[[0, P], [1, N_EXP]]),
            )

            for it in [t for _ in range(repeat) for t in range(NTILES)]:
                x_sb = xin.tile([P, IN_F], fp32, tag="x")
                nc.sync.dma_start(x_sb, x_t[it])

                # transpose x tile -> [i, t] chunks via PE (exact fp32)
                if "transpose" in ablate:
                    xt_sb = xtp.tile([P, KC, P], fp32, tag="xt")
                    nc.scalar.copy(xt_sb, x_sb.rearrange("p (c q) -> p c q", c=KC))
                else:
                    pxt = ps_xt.tile([P, KC, P], fp32, tag="pxt")
                    for c in range(KC):
                        nc.tensor.transpose(pxt[:, c, :],
                                            x_sb[:, c * P:(c + 1) * P], ident)
                    xt_sb = xtp.tile([P, KC, P], fp32, tag="xt")
                    nc.scalar.copy(xt_sb, pxt)
                # fp32r-rounded copy for the base/expert matmuls (DVE 2x mode)
                xt_r = xtp.tile([P, KC, P], f32r, tag="xtr")
                nc.vector.tensor_copy(xt_r, xt_sb)

                if "router" in ablate:
                    w_sb = smalls.tile([P, N_EXP], fp32, tag="w")
                    nc.vector.memset(w_sb, 0.25)
                else:
                    # router logits in exact fp32 (argmax must match reference)
                    psm = ps_sm.tile([P, 132], fp32, tag="psm")
                    logits_ps = psm[:, 0:N_EXP]
                    for c in range(KC):
                        nc.tensor.matmul(logits_ps, xt_sb[:, c, :],
                                         wrt_sb[:, c, :],
                                         start=(c == 0), stop=(c == KC - 1))

                    # softmax top-1: w[t,e] = (logit==max) * 1/sum(exp(l-max))
                    lg = smalls.tile([P, N_EXP], fp32, tag="lg")
                    nc.vector.tensor_tensor(lg, logits_ps, brp_sb, Alu.add)
                    m = smalls.tile([P, 1], fp32, tag="m")
                    nc.vector.tensor_reduce(m, lg, axis=Ax.X, op=Alu.max)
                    negm = smalls.tile([P, 1], fp32, tag="negm")
                    nc.vector.tensor_scalar_mul(negm, m, -1.0)
                    ex = smalls.tile([P, N_EXP], fp32, tag="ex")
                    nc.scalar.activation(ex, lg, Act.Exp, bias=negm, scale=1.0)
                    ssum = smalls.tile([P, 1], fp32, tag="ssum")
                    nc.vector.tensor_reduce(ssum, ex, axis=Ax.X, op=Alu.add)
                    rs = smalls.tile([P, 1], fp32, tag="rs")
                    nc.vector.reciprocal(rs, ssum)
                    w_sb = smalls.tile([P, N_EXP], fp32, tag="w")
                    nc.vector.tensor_scalar(w_sb, lg, m, rs, Alu.is_equal,
                                            Alu.mult)

                # base: x@V + bbp + sum_e w_e*be[e]
                pb = ps_base.tile([P, OUT_F], fp32, tag="pb")
                if "base" in ablate:
                    nc.tensor.matmul(pb, xt_r[:, 0, :], vt_sb[:, 0, :],
                                     start=True, stop=True)
                else:
                    for c in range(KC):
                        nc.tensor.matmul(pb, xt_r[:, c, :], vt_sb[:, c, :],
                                         start=(c == 0), stop=False)
                    nc.tensor.matmul(pb, ones_r, bbp_sb,
                                     start=False, stop=False)
                    if "router" in ablate or "bemm" in ablate:
                        nc.tensor.matmul(pb, ones_r, bbp_sb,
                                         start=False, stop=True)
                    else:
                        # wT [4, t] for the be-combine matmul
                        wT_ps = psm[:N_EXP, N_EXP:N_EXP + P]
                        nc.tensor.transpose(wT_ps, w_sb, ident)
                        wT_sb = smalls.tile([N_EXP, P], f32r, tag="wT")
                        nc.scalar.copy(wT_sb, wT_ps)
                        nc.tensor.matmul(pb, wT_sb, bep_sb,
                                         start=False, stop=True)

                # experts (fp32r)
                n_exp_run = 0 if "experts" in ablate else N_EXP
                pe_tiles = []
                for e in range(n_exp_run):
                    pe_t = ps_exp.tile([P, OUT_F], fp32, tag=f"exp{e}")
                    for c in range(KC):
                        nc.tensor.matmul(pe_t, xt_r[:, c, :],
                                         wet_sb[:, c, e, :],
                                         start=(c == 0), stop=(c == KC - 1))
                    pe_tiles.append(pe_t)

                # combine: acc = base + sum_e w_e * y_e
                acc = outp.tile([P, OUT_F], fp32, tag="acc")
                nc.scalar.copy(acc, pb)
                if "combine" not in ablate:
                    for e in range(n_exp_run):
                        nc.vector.scalar_tensor_tensor(
                            acc, pe_tiles[e], w_sb[:, e:e + 1], acc,
                            Alu.mult, Alu.add)
                nc.sync.dma_start(out_t[it], acc)

    nc.compile()
    return nc


def _get_nc():
    global _compiled_nc
    if _compiled_nc is None:
        _compiled_nc = _build()
    return _compiled_nc


def _round_tf32(a):
    """Round fp32 array to tf32 (10-bit mantissa), round-to-nearest-even."""
    u = np.ascontiguousarray(a, np.float32).view(np.uint32)
    bias = ((u >> np.uint32(13)) & np.uint32(1)) + np.uint32(0x0FFF)
    u = (u + bias) & np.uint32(0xFFFFE000)
    return u.view(np.float32)


def _prep_weights(inputs):
    f64 = np.float64
    Wb = np.asarray(inputs["Wb"], f64)
    bb = np.asarray(inputs["bb"], f64)
    A = np.asarray(inputs["A"], f64)
    Bm = np.asarray(inputs["B"], f64)
    ia3 = np.asarray(inputs["ia3"], f64)
    Wr = np.asarray(inputs["Wr"], np.float32)
    br = np.asarray(inputs["br"], np.float32)
    We = np.asarray(inputs["We"], np.float32)
    be = np.asarray(inputs["be"], np.float32)

    V = ((Wb.T + (ALPHA / RANK) * (A @ Bm)) * ia3[None, :]).astype(np.float32)
    # device layouts: [p, c, ...] with input feature i = c*128 + p
    vt_l = V.reshape(KC, P, OUT_F).transpose(1, 0, 2)
    wet_l = (We.transpose(2, 0, 1)  # [i, e, o]
             .reshape(KC, P, N_EXP, OUT_F).transpose(1, 0, 2, 3))
    wrt_l = Wr.T.reshape(KC, P, N_EXP).transpose(1, 0, 2)
    return {
        "vt": _round_tf32(vt_l),
        "bbp": _round_tf32((bb * ia3).astype(np.float32).reshape(1, OUT_F)),
        "wet": _round_tf32(wet_l),
        "bep": _round_tf32(be),
        "wrt": np.ascontiguousarray(wrt_l),
        "brp": np.ascontiguousarray(br.reshape(1, N_EXP)),
        "ones": np.ones((1, P), np.float32),
    }


def _run(in_maps, trace=False):
    from concourse.bass_utils import run_bass_kernel_spmd
    nc = _get_nc()
    return run_bass_kernel_spmd(nc, in_maps, list(range(N_CORES)), trace=trace)


def _make_in_maps(inputs):
    x = np.asarray(inputs["x"], np.float32)
    assert x.shape == (B, S, IN_F)
    w = _prep_weights(inputs)
    return [dict(x=np.ascontiguousarray(x[c]), **w) for c in range(N_CORES)]


def kernel(**inputs):
    in_maps = _make_in_maps(inputs)
    res = _run(in_maps)
    outs = [np.asarray(res.results[c]["out"]) for c in range(N_CORES)]
    return np.stack(outs, axis=0).astype(np.float32)


# revision 22
# speedup vs baseline: 3.0115x; 3.0115x over previous
"""Trainium2 Bass kernel for nn_PolyAdapter (base Linear + LoRA + IA3 + dense top-1 MoE).

Math (per token row x of length 512):
  out = (x @ Wb.T + bb + 4*(x@A)@B) * ia3 + w_sel * (x @ We[sel].T + be[sel])
where sel = argmax softmax(x @ Wr.T + br), w_sel its softmax weight.

Host folds LoRA + IA3 into one base matrix:  V = (Wb.T + 4*A@B) * ia3[None,:],
bbp = bb*ia3.  Device computes, per 128-token tile:
  - PE transpose of x tile (fp32, exact) to get features-on-partitions
  - router logits in fp32 (exact argmax vs reference), softmax top-1 mask on DVE
  - base psum = x@V (+ bbp via ones-row matmul, + sum_e w_e*be[e] via wT matmul)
  - 4 expert matmuls (fp32r), combined with per-token weights via fused
    scalar_tensor_tensor ops on DVE.

Sharding: data-parallel, batch row b -> core b (8 cores, 8192 tokens each).
"""

import numpy as np

IN_F = 512
OUT_F = 512
RANK = 8
ALPHA = 32.0
N_EXP = 4
B = 8
S = 8192
P = 128
N_CORES = 8
NTILES = S // P  # 64 tiles of 128 tokens per core
KC = IN_F // P  # 4 contraction chunks

_compiled_nc = None


def _build(repeat=1, ablate=("dvebias",)):
    import concourse.bass as bass
    import concourse.tile as tile
    from concourse import bacc, mybir
    from concourse.masks import make_identity

    fp32 = mybir.dt.float32
    f32r = mybir.dt.float32r
    Alu = mybir.AluOpType
    Act = mybir.ActivationFunctionType
    Ax = mybir.AxisListType

    nc = bacc.Bacc("TRN2", target_bir_lowering=False, debug=False,
                   enable_asserts=False)

    # weights arrive pre-arranged in the exact SBUF layouts (see _prep_weights);
    # fp32r (tf32) operands are pre-rounded on the host
    x = nc.dram_tensor("x", [S, IN_F], fp32, kind="ExternalInput").ap()
    vt = nc.dram_tensor("vt", [P, KC, OUT_F], f32r, kind="ExternalInput").ap()
    bbp = nc.dram_tensor("bbp", [1, OUT_F], f32r, kind="ExternalInput").ap()
    wet = nc.dram_tensor("wet", [P, KC, N_EXP, OUT_F], f32r, kind="ExternalInput").ap()
    bep = nc.dram_tensor("bep", [N_EXP, OUT_F], f32r, kind="ExternalInput").ap()
    wrt = nc.dram_tensor("wrt", [P, KC, N_EXP], fp32, kind="ExternalInput").ap()
    brp = nc.dram_tensor("brp", [1, N_EXP], fp32, kind="ExternalInput").ap()
    ones = nc.dram_tensor("ones", [1, P], f32r, kind="ExternalInput").ap()
    out = nc.dram_tensor("out", [S, OUT_F], fp32, kind="ExternalOutput").ap()

    x_t = x.rearrange("(n p) i -> n p i", p=P)
    out_t = out.rearrange("(n p) o -> n p o", p=P)

    with tile.TileContext(nc) as tc:
        with (
            tc.tile_pool(name="singles", bufs=1) as singles,
            tc.tile_pool(name="xin", bufs=4) as xin,
            tc.tile_pool(name="xtp", bufs=3) as xtp,
            tc.tile_pool(name="smalls", bufs=3) as smalls,
            tc.tile_pool(name="outp", bufs=4) as outp,
            tc.tile_pool(name="ps_xt", bufs=1, space="PSUM") as ps_xt,
            tc.tile_pool(name="ps_sm", bufs=1, space="PSUM") as ps_sm,
            tc.tile_pool(name="ps_base", bufs=2, space="PSUM") as ps_base,
            tc.tile_pool(name="ps_exp", bufs=1, space="PSUM") as ps_exp,
        ):
            ident = singles.tile([P, P], fp32)
            make_identity(nc, ident)
            ones_r = singles.tile([1, P], f32r)
            nc.sync.dma_start(ones_r, ones)

            vt_sb = singles.tile([P, KC, OUT_F], f32r)
            nc.sync.dma_start(vt_sb, vt)
            wet_sb = singles.tile([P, KC, N_EXP, OUT_F], f32r)
            nc.sync.dma_start(wet_sb, wet)
            wrt_sb = singles.tile([P, KC, N_EXP], fp32)
            nc.sync.dma_start(wrt_sb, wrt)
            bbp_sb = singles.tile([1, OUT_F], f32r)
            nc.sync.dma_start(bbp_sb, bbp)
            bep_sb = singles.tile([N_EXP, OUT_F], f32r)
            nc.sync.dma_start(bep_sb, bep)
            brp_sb = singles.tile([P, N_EXP], fp32)
            nc.gpsimd.dma_start(
                brp_sb,
                bass.AP(tensor=brp.tensor, offset=brp.offset,
                        ap=[[0, P], [1, N_EXP]]),
            )
            if "dvebias" in ablate:
                bbp_bc = singles.tile([P, OUT_F], fp32)
                nc.gpsimd.dma_start(
                    bbp_bc,
                    bass.AP(tensor=bbp.tensor, offset=bbp.offset,
                            ap=[[0, P], [1, OUT_F]]),
                )

            for it in [t for _ in range(repeat) for t in range(NTILES)]:
                x_sb = xin.tile([P, IN_F], fp32, tag="x")
                nc.sync.dma_start(x_sb, x_t[it])

                # transpose x tile -> [i, t] chunks via PE (exact fp32)
                if "transpose" in ablate:
                    xt_sb = xtp.tile([P, KC, P], fp32, tag="xt")
                    nc.scalar.copy(xt_sb, x_sb.rearrange("p (c q) -> p c q", c=KC))
                else:
                    pxt = ps_xt.tile([P, KC, P], fp32, tag="pxt")
                    for c in range(KC):
                        nc.tensor.transpose(pxt[:, c, :],
                                            x_sb[:, c * P:(c + 1) * P], ident)
                    xt_sb = xtp.tile([P, KC, P], fp32, tag="xt")
                    nc.scalar.copy(xt_sb, pxt)
                # fp32r-rounded copy for the base/expert matmuls (DVE 2x mode)
                xt_r = xtp.tile([P, KC, P], f32r, tag="xtr")
                nc.vector.tensor_copy(xt_r, xt_sb)

                if "router" in ablate:
                    w_sb = smalls.tile([P, N_EXP], fp32, tag="w")
                    nc.vector.memset(w_sb, 0.25)
                else:
                    # router logits in exact fp32 (argmax must match reference)
                    psm = ps_sm.tile([P, 132], fp32, tag="psm")
                    logits_ps = psm[:, 0:N_EXP]
                    for c in range(KC):
                        nc.tensor.matmul(logits_ps, xt_sb[:, c, :],
                                         wrt_sb[:, c, :],
                                         start=(c == 0), stop=(c == KC - 1))

                    # softmax top-1: w[t,e] = (logit==max) * 1/sum(exp(l-max))
                    lg = smalls.tile([P, N_EXP], fp32, tag="lg")
                    nc.vector.tensor_tensor(lg, logits_ps, brp_sb, Alu.add)
                    m = smalls.tile([P, 1], fp32, tag="m")
                    nc.vector.tensor_reduce(m, lg, axis=Ax.X, op=Alu.max)
                    negm = smalls.tile([P, 1], fp32, tag="negm")
                    nc.vector.tensor_scalar_mul(negm, m, -1.0)
                    ex = smalls.tile([P, N_EXP], fp32, tag="ex")
                    nc.scalar.activation(ex, lg, Act.Exp, bias=negm, scale=1.0)
                    ssum = smalls.tile([P, 1], fp32, tag="ssum")
                    nc.vector.tensor_reduce(ssum, ex, axis=Ax.X, op=Alu.add)
                    rs = smalls.tile([P, 1], fp32, tag="rs")
                    nc.vector.reciprocal(rs, ssum)
                    w_sb = smalls.tile([P, N_EXP], fp32, tag="w")
                    nc.vector.tensor_scalar(w_sb, lg, m, rs, Alu.is_equal,
                                            Alu.mult)

                # base: x@V (+ bbp) + sum_e w_e*be[e]
                pb = ps_base.tile([P, OUT_F], fp32, tag="pb")
                for c in range(KC):
                    nc.tensor.matmul(pb, xt_r[:, c, :], vt_sb[:, c, :],
                                     start=(c == 0), stop=False)
                if "dvebias" not in ablate:
                    nc.tensor.matmul(pb, ones_r, bbp_sb,
                                     start=False, stop=False)
                # wT [4, t] for the be-combine matmul
                wT_ps = psm[:N_EXP, N_EXP:N_EXP + P]
                nc.tensor.transpose(wT_ps, w_sb, ident)
                wT_sb = smalls.tile([N_EXP, P], f32r, tag="wT")
                nc.scalar.copy(wT_sb, wT_ps)
                nc.tensor.matmul(pb, wT_sb, bep_sb,
                                 start=False, stop=True)

                if "half" in ablate:
                    # o-halved expert psums: 4 experts x 2 halves of 256,
                    # half-bank tiles -> double-buffered pools fit in PSUM
                    acc = outp.tile([P, OUT_F], fp32, tag="acc")
                    if "dvebias" in ablate:
                        nc.vector.tensor_tensor(acc, pb, bbp_bc, Alu.add)
                    else:
                        nc.scalar.copy(acc, pb)
                    H = OUT_F // 2
                    for e in range(N_EXP):
                        for h in range(2):
                            pe_t = ps_exp.tile([P, H], fp32, tag=f"exp{e}h{h}")
                            for c in range(KC):
                                nc.tensor.matmul(
                                    pe_t, xt_r[:, c, :],
                                    wet_sb[:, c, e, h * H:(h + 1) * H],
                                    start=(c == 0), stop=(c == KC - 1))
                            nc.vector.scalar_tensor_tensor(
                                acc[:, h * H:(h + 1) * H], pe_t,
                                w_sb[:, e:e + 1], acc[:, h * H:(h + 1) * H],
                                Alu.mult, Alu.add)
                else:
                    pe_tiles = []
                    for e in range(N_EXP):
                        pe_t = ps_exp.tile([P, OUT_F], fp32, tag=f"exp{e}")
                        for c in range(KC):
                            nc.tensor.matmul(pe_t, xt_r[:, c, :],
                                             wet_sb[:, c, e, :],
                                             start=(c == 0), stop=(c == KC - 1))
                        pe_tiles.append(pe_t)

                    # combine: acc = base (+bbp) + sum_e w_e * y_e
                    acc = outp.tile([P, OUT_F], fp32, tag="acc")
                    if "dvebias" in ablate:
                        nc.vector.tensor_tensor(acc, pb, bbp_bc, Alu.add)
                    else:
                        nc.scalar.copy(acc, pb)
                    for e in range(N_EXP):
                        nc.vector.scalar_tensor_tensor(
                            acc, pe_tiles[e], w_sb[:, e:e + 1], acc,
                            Alu.mult, Alu.add)
                nc.sync.dma_start(out_t[it], acc)

    nc.compile()
    return nc


def _get_nc():
    global _compiled_nc
    if _compiled_nc is None:
        _compiled_nc = _build()
    return _compiled_nc


def _round_tf32(a):
    """Round fp32 array to tf32 (10-bit mantissa), round-to-nearest-even."""
    u = np.ascontiguousarray(a, np.float32).view(np.uint32)
    bias = ((u >> np.uint32(13)) & np.uint32(1)) + np.uint32(0x0FFF)
    u = (u + bias) & np.uint32(0xFFFFE000)
    return u.view(np.float32)


def _prep_weights(inputs):
    f64 = np.float64
    Wb = np.asarray(inputs["Wb"], f64)
    bb = np.asarray(inputs["bb"], f64)
    A = np.asarray(inputs["A"], f64)
    Bm = np.asarray(inputs["B"], f64)
    ia3 = np.asarray(inputs["ia3"], f64)
    Wr = np.asarray(inputs["Wr"], np.float32)
    br = np.asarray(inputs["br"], np.float32)
    We = np.asarray(inputs["We"], np.float32)
    be = np.asarray(inputs["be"], np.float32)

    V = ((Wb.T + (ALPHA / RANK) * (A @ Bm)) * ia3[None, :]).astype(np.float32)
    # device layouts: [p, c, ...] with input feature i = c*128 + p
    vt_l = V.reshape(KC, P, OUT_F).transpose(1, 0, 2)
    wet_l = (We.transpose(2, 0, 1)  # [i, e, o]
             .reshape(KC, P, N_EXP, OUT_F).transpose(1, 0, 2, 3))
    wrt_l = Wr.T.reshape(KC, P, N_EXP).transpose(1, 0, 2)
    return {
        "vt": _round_tf32(vt_l),
        "bbp": _round_tf32((bb * ia3).astype(np.float32).reshape(1, OUT_F)),
        "wet": _round_tf32(wet_l),
        "bep": _round_tf32(be),
        "wrt": np.ascontiguousarray(wrt_l),
        "brp": np.ascontiguousarray(br.reshape(1, N_EXP)),
        "ones": np.ones((1, P), np.float32),
    }


def _run(in_maps, trace=False):
    from concourse.bass_utils import run_bass_kernel_spmd
    nc = _get_nc()
    return run_bass_kernel_spmd(nc, in_maps, list(range(N_CORES)), trace=trace)


def _make_in_maps(inputs):
    x = np.asarray(inputs["x"], np.float32)
    assert x.shape == (B, S, IN_F)
    w = _prep_weights(inputs)
    return [dict(x=np.ascontiguousarray(x[c]), **w) for c in range(N_CORES)]


def kernel(**inputs):
    in_maps = _make_in_maps(inputs)
    res = _run(in_maps)
    outs = [np.asarray(res.results[c]["out"]) for c in range(N_CORES)]
    return np.stack(outs, axis=0).astype(np.float32)
